# revision 10
# baseline (speedup 1.0000x reference)
"""BarrierNet (MLP 4->512->{128,128}->{2,2} + closed-form QP) on 8 Trainium2 cores.

Data-parallel: batch 262144 sharded 8 x 32768; weights replicated.

Per-core layout: sample s = p*256 + 4t + j (p = SBUF partition, t = 512-sample
tile, j = 0..3). The MLP runs feature-major (batch on the PE free dim):
L1 is fp32r with K=4 (b1 folded into the PSUM->SBUF ReLU as a per-partition
bias); its output h1 is written to SBUF as fp8e4 split across ACT/DVE.
L2 runs fp8e4 DoubleRow matmuls (2 K-tiles per instruction, 0.5 cycles/row):
16 matmuls of out [64, 256] contract K=512 into h21/h22 PSUM. h21/h22 are
drained to SBUF fp32 with bias+ReLU. L3 is flipped: the h2 128-sample chunk
is the *stationary* operand and the stacked W31/W32 columns are the moving
[128, 4] operand, so each matmul writes out [128 samples, 4] directly in
sample-major order into a per-sp PSUM bank (DVE memset once, start=False
accumulation). That removes the PE transposes, the staging copies and the
DRAM bounce of the old pipeline entirely. The QP (sin/cos via range-reduced
ACT Sin, sigmoid, one reciprocal) runs as [128, 128] ops per sp half on
Pool/DVE/ACT, reading px/py/theta/v from x_nat (sample-major by construction).
QP ops are emitted as closure lists drained a couple per pipeline slot so no
engine sees a burst that would stall the per-tile critical path.
"""
import numpy as np
from contextlib import ExitStack

import ml_dtypes

import concourse.bass as bass
from concourse import bacc as bacc_mod
import concourse.tile as tile
from concourse import mybir
from concourse.bass_utils import run_bass_kernel_spmd

F32 = mybir.dt.float32
F32R = mybir.dt.float32r
F8 = mybir.dt.float8e4
AF = mybir.ActivationFunctionType
OP = mybir.AluOpType
DR = mybir.MatmulPerfMode.DoubleRow

MAGIC = float(np.float32(1.5 * 2 ** 23))
INV2PI = float(np.float32(1.0 / (2 * np.pi)))
TWOPI = float(np.float32(2 * np.pi))
HALFPI = float(np.float32(np.pi / 2))

N_CORES = 8
NB = 262144
B = NB // N_CORES  # 32768 per core

_CACHE = {}


def _emit(nc, tc, ctx, aps, B):
    (xT4, x_nat, w1r, w2s, w3s, b1s, b2s, b3bc, u_out) = aps
    T = B // 512           # 64 sample tiles per core
    NS = T // 32           # 2 QP half-batches ("sp")

    const = ctx.enter_context(tc.tile_pool(name="const", bufs=1))
    lp = ctx.enter_context(tc.tile_pool(name="lp", bufs=1))
    qp = ctx.enter_context(tc.tile_pool(name="qp", bufs=1))
    ps = ctx.enter_context(tc.tile_pool(name="ps", bufs=1, space="PSUM"))

    w1_sb = const.tile([128, 512], F32R, name="w1_sb", tag="w1_sb")
    nc.sync.dma_start(w1_sb[:], w1r[:])
    w2_sb = const.tile([128, 2, 2, 2, 2, 64], F8, name="w2_sb", tag="w2_sb")
    nc.sync.dma_start(w2_sb[:], w2s[:])
    w3_sb = const.tile([64, 2, 2, 4], F32R, name="w3_sb", tag="w3_sb")
    nc.sync.dma_start(w3_sb[:], w3s[:])
    b1_sb = const.tile([128, 4], F32, name="b1_sb", tag="b1_sb")
    nc.sync.dma_start(b1_sb[:], b1s[:])
    b2_sb = const.tile([64, 2, 2], F32, name="b2_sb", tag="b2_sb")
    nc.sync.dma_start(b2_sb[:], b2s[:])
    b3_sb = const.tile([128, 4], F32, name="b3_sb", tag="b3_sb")
    nc.sync.dma_start(b3_sb[:], b3bc[:])
    xn_sb = const.tile([128, NS, 128, 4], F32, name="xn_sb", tag="xn_sb")
    nc.sync.dma_start(xn_sb[:], x_nat[:])
    # X4: tile t lives at partition base 32*(t%4), columns 512*(t//4)..
    x4_sb = const.tile([128, T // 4, 512], F32R, name="x4_sb", tag="x4_sb")
    xv = xT4.rearrange("q (tt c r) -> q tt c r", c=4, r=512)
    for c in range(4):
        nc.sync.dma_start(x4_sb[32 * c:32 * c + 4, :, :], xv[:, :, c, :])

    # software-pipelined: window w runs L1(w), L2(w-1), L3(w-2), QP spread
    h1s = {}
    h2s = {}
    ps3s = {}
    geo = {}
    pend = []  # queue of QP op closures, drained a few per slot
    def emit_l1(t, blocks):
        cb = 32 * (t % 4)
        xc = x4_sb[cb:cb + 4, t // 4, :]
        h1, pbs = h1s[t]
        for b in blocks:
            psb = ps.tile([128, 512], F32, name="ps1", tag="ps1", bufs=3)
            nc.tensor.matmul(
                psb[:], w1_sb[cb:cb + 4, 128 * b:128 * (b + 1)],
                xc, start=True, stop=True, tile_position=(cb, 0))
            if b < 3:
                nc.scalar.activation(h1[:, b, :], psb[:], AF.Relu,
                                     bias=b1_sb[:, b:b + 1])
            else:
                nc.vector.tensor_scalar(h1[:, b, :], psb[:],
                                        b1_sb[:, b:b + 1], 0.0,
                                        op0=OP.add, op1=OP.max)

    def emit_l2_hh(t, hh):
        h1 = h1s[t][0]
        h21, h22 = h2s[t]
        # DoubleRow writes PSUM partitions 0:64 only; hidden block m goes to
        # bank m. h2 stays in hidden-half layout [64, m, samples]; L3
        # contracts the two halves as separate K=64 chunks.
        ps2 = ps.tile([64, 2, 2, 256], F32, name="ps2", tag="ps2", bufs=2)
        nsl = slice(256 * hh, 256 * hh + 256)
        for m in range(2):
            for o in range(2):
                for kp in range(2):
                    nc.tensor.matmul(
                        ps2[0:64, m, o, :],
                        w2_sb[:, o, kp, m, :, :],
                        h1[:, 2 * kp:2 * kp + 2, nsl],
                        start=(o == 0 and kp == 0), stop=(kp == 1),
                        perf_mode=DR, skip_group_check=True)
        for m in range(2):
            for o in range(2):
                dst = (h21 if o == 0 else h22)[0:64, m, nsl]
                src = ps2[0:64, m, o, :]
                bias = b2_sb[:, m, o:o + 1]
                if (hh, m, o) in ((0, 0, 0), (0, 1, 0), (1, 0, 0)):
                    nc.scalar.activation(dst, src, AF.Relu, bias=bias)
                else:
                    nc.vector.tensor_scalar(dst, src, bias, 0.0,
                                            op0=OP.add, op1=OP.max)

    for w in range(T + 4):
        if w < T:
            t = w
            h1s[t] = (lp.tile([128, 4, 512], F8, name="h1", tag="h1",
                              bufs=3), None)
            h2s[t] = (lp.tile([64, 2, 512], F32R, name="h21", tag="h21",
                              bufs=2),
                      lp.tile([64, 2, 512], F32R, name="h22", tag="h22",
                              bufs=2))
            emit_l1(t, (0, 1, 2))
        if 1 <= w <= T:
            emit_l2_hh(w - 1, 0)
        if w < T:
            emit_l1(w, (3,))
        if 1 <= w <= T:
            emit_l2_hh(w - 1, 1)
            h1s.pop(w - 1)
        if 2 <= w <= T + 1:
            t = w - 2
            sp, g = divmod(t, 32)
            if g == 0:
                ps3 = ps.tile([128, 128, 4], F32, name="ps3", tag="ps3",
                              bufs=1)
                ps3s[sp] = ps3
                nc.vector.memset(ps3[:], 0.0)
                geo[sp] = {}
                pend.extend(_qp_geo_ops(nc, qp, xn_sb, sp, geo[sp]))
            ps3 = ps3s[sp]
            h21, h22 = h2s.pop(t)
            for j in range(4):
                mi = 4 * g + j
                for m in range(2):
                    nc.tensor.matmul(
                        ps3[:, mi, :], h21[0:64, m, 128 * j:128 * (j + 1)],
                        w3_sb[0:64, 0, m, :], start=False, stop=False,
                        skip_group_check=True)
                    nc.tensor.matmul(
                        ps3[:, mi, :], h22[0:64, m, 128 * j:128 * (j + 1)],
                        w3_sb[0:64, 1, m, :], start=False,
                        stop=(g == 31 and j == 3 and m == 1),
                        skip_group_check=True)
            if g == 31:
                s3 = qp.tile([128, 128, 4], F32, name="s3", tag="s3", bufs=2)
                nc.scalar.copy(s3[:], ps3s.pop(sp)[:])
                pend.extend(
                    _qp_rest_ops(nc, qp, s3, b3_sb, u_out, sp, geo.pop(sp)))
        # drain a few pending QP ops per slot to avoid engine bursts
        for _ in range(3):
            if pend:
                pend.pop(0)()
    while pend:
        pend.pop(0)()


def _qp_tile(nc, qp, name, bufs=1):
    return qp.tile([128, 128], F32, name=name, tag=name, bufs=bufs)


def _qp_geo_ops(nc, qp, xn_sb, sp, out):
    """x-only QP quantities (no MLP dependency) as a list of op closures.

    Results consumed by _qp_rest_ops (next sp window) use bufs=2 tiles.
    """
    r = {}

    def tt(name, a, b, op, eng=None, bufs=1):
        def f():
            o = _qp_tile(nc, qp, name, bufs=bufs)
            (eng or nc.gpsimd).tensor_tensor(o[:], r[a], r[b], op=op)
            r[name] = o[:]
            out[name] = o[:]
        return f

    def ts(name, a, s1, op0, s2=None, op1=None, eng=None, bufs=1):
        def f():
            o = _qp_tile(nc, qp, name, bufs=bufs)
            if s2 is None:
                (eng or nc.vector).tensor_scalar(o[:], r[a], s1, None, op0=op0)
            else:
                (eng or nc.vector).tensor_scalar(o[:], r[a], s1, s2,
                                                 op0=op0, op1=op1)
            r[name] = o[:]
            out[name] = o[:]
        return f

    def act(name, a, func):
        def f():
            o = _qp_tile(nc, qp, name)
            nc.scalar.activation(o[:], r[a], func)
            r[name] = o[:]
            out[name] = o[:]
        return f

    r["PX"] = xn_sb[:, sp, :, 0]
    r["PY"] = xn_sb[:, sp, :, 1]
    r["TH"] = xn_sb[:, sp, :, 2]
    r["VV"] = xn_sb[:, sp, :, 3]

    V = nc.vector
    P = nc.gpsimd
    return [fn for fn in [
        ts("dx", "PX", 10.0, OP.mult, 10.0, OP.add),
        ts("dy", "PY", 10.0, OP.mult, 5.0, OP.add),
        ts("v", "VV", 2.0, OP.mult, 5.0, OP.add),
        ts("f1", "TH", INV2PI, OP.mult, MAGIC, OP.add),
        ts("f2", "f1", MAGIC, OP.subtract, TWOPI, OP.mult),
        tt("thr", "TH", "f2", OP.subtract, eng=P),
        act("st", "thr", AF.Sin),
        ts("c1", "TH", INV2PI, OP.mult, 0.25, OP.add),
        ts("c2", "c1", MAGIC, OP.add, MAGIC, OP.subtract),
        ts("c3", "c2", TWOPI, OP.mult, HALFPI, OP.subtract),
        tt("thc", "TH", "c3", OP.subtract, eng=P),
        act("ct", "thc", AF.Sin),
        tt("vst", "v", "st", OP.mult, eng=P),
        tt("vct", "v", "ct", OP.mult, eng=P),
        tt("dx2", "dx", "dx", OP.mult, eng=P),
        tt("dy2", "dy", "dy", OP.mult, eng=P),
        tt("bar", "dx2", "dy2", OP.add, eng=P),
        ts("bar16", "bar", 16.0, OP.mult, 576.0, OP.subtract, bufs=2),
        tt("a3", "dx", "vct", OP.mult, eng=P),
        tt("a4", "dy", "vst", OP.mult, eng=P),
        tt("a5", "a3", "a4", OP.add, eng=P),
        ts("bdot4", "a5", 8.0, OP.mult, bufs=2),
        tt("v2", "v", "v", OP.mult, eng=P),
        ts("v22", "v2", 2.0, OP.mult, bufs=2),
        tt("g1a", "dx", "vst", OP.mult, eng=P),
        tt("g1b", "dy", "vct", OP.mult, eng=P),
        tt("G1p", "g1a", "g1b", OP.subtract, eng=P, bufs=2),
        tt("g2a", "dx", "ct", OP.mult, eng=P),
        tt("g2b", "dy", "st", OP.mult, eng=P),
        tt("G2pp", "g2a", "g2b", OP.add, eng=P, bufs=2),
        tt("q1", "G1p", "G1p", OP.mult, eng=P),
        tt("q2", "G2pp", "G2pp", OP.mult, eng=P),
        tt("q3", "q1", "q2", OP.add, eng=P),
        ts("ggc", "q3", 4.0, OP.mult, 1e-12, OP.max),
        _recip(nc, qp, r, out),
    ] if fn is not None]


def _recip(nc, qp, r, out):
    def f():
        o = _qp_tile(nc, qp, "rec", bufs=2)
        nc.vector.reciprocal(o[:], r["ggc"])
        out["rec"] = o[:]
    return f


def _qp_rest_ops(nc, qp, s3, b3_sb, u_out, sp, g):
    r = dict(g)

    def tt(name, a, b, op, eng=None):
        def f():
            o = _qp_tile(nc, qp, name)
            (eng or nc.gpsimd).tensor_tensor(o[:], r[a], r[b], op=op)
            r[name] = o[:]
        return f

    def ts(name, a, s1, op0, eng=None):
        def f():
            o = _qp_tile(nc, qp, name)
            (eng or nc.vector).tensor_scalar(o[:], r[a], s1, None, op0=op0)
            r[name] = o[:]
        return f

    def stt(name, a, s, b, op0, op1, eng=None):
        def f():
            o = _qp_tile(nc, qp, name)
            (eng or nc.vector).scalar_tensor_tensor(o[:], r[a], s, r[b],
                                                    op0=op0, op1=op1)
            r[name] = o[:]
        return f

    def sig(name, src, bcol):
        def f():
            o = _qp_tile(nc, qp, name)
            nc.scalar.activation(o[:], src, AF.Sigmoid,
                                 bias=b3_sb[:, bcol:bcol + 1])
            r[name] = o[:]
        return f

    def addb(name, src, bcol):
        def f():
            o = _qp_tile(nc, qp, name)
            nc.vector.tensor_scalar(o[:], src, b3_sb[:, bcol:bcol + 1], None,
                                    op0=OP.add)
            r[name] = o[:]
        return f

    X31A = s3[:, :, 0]
    X31B = s3[:, :, 1]
    Z32A = s3[:, :, 2]
    Z32B = s3[:, :, 3]

    def emit_u():
        u_sb = qp.tile([128, 128, 2], F32, name="u_sb", tag="u_sb", bufs=2)
        nc.gpsimd.tensor_tensor(u_sb[:, :, 0], r["z1"], r["xa"],
                                op=OP.subtract)
        nc.vector.tensor_tensor(u_sb[:, :, 1], r["z2"], r["xb"],
                                op=OP.subtract)
        nc.sync.dma_start(u_out[:, sp, :, :], u_sb[:])

    return [
        sig("sa", Z32A, 2),
        sig("sb_", Z32B, 3),
        addb("xa", X31A, 0),
        addb("xb", X31B, 1),
        tt("ssum", "sa", "sb_", OP.add),
        tt("sprod", "sa", "sb_", OP.mult),
        tt("m1", "ssum", "bdot4", OP.mult),
        tt("m2", "sprod", "bar16", OP.mult),
        tt("m3", "m1", "m2", OP.add),
        tt("h", "v22", "m3", OP.add),
        tt("n1", "G1p", "xa", OP.mult),
        tt("n2", "G2pp", "xb", OP.mult),
        tt("n3", "n1", "n2", OP.subtract, eng=nc.vector),
        stt("num", "n3", -2.0, "h", OP.mult, OP.subtract),
        stt("lam", "num", 0.0, "rec", OP.max, OP.mult),
        stt("z1", "lam", -2.0, "G1p", OP.mult, OP.mult),
        stt("z2", "lam", 2.0, "G2pp", OP.mult, OP.mult),
        emit_u,
    ]


def _build_kernel(n_cores, B):
    nc = bacc_mod.Bacc("TRN2", target_bir_lowering=False, debug=False,
                       num_devices=n_cores)
    NS = (B // 512) // 32
    xT4 = nc.dram_tensor("xT4", [4, B], F32R, kind="ExternalInput").ap()
    x_nat = nc.dram_tensor("x_nat", [128, NS, 128, 4], F32,
                           kind="ExternalInput").ap()
    w1r = nc.dram_tensor("w1r", [128, 512], F32R, kind="ExternalInput").ap()
    w2s = nc.dram_tensor("w2s", [128, 2, 2, 2, 2, 64], F8,
                         kind="ExternalInput").ap()
    w3s = nc.dram_tensor("w3s", [64, 2, 2, 4], F32R,
                         kind="ExternalInput").ap()
    b1s = nc.dram_tensor("b1s", [128, 4], F32, kind="ExternalInput").ap()
    b2s = nc.dram_tensor("b2s", [64, 2, 2], F32, kind="ExternalInput").ap()
    b3bc = nc.dram_tensor("b3bc", [128, 4], F32, kind="ExternalInput").ap()
    u_out = nc.dram_tensor("u_out", [128, NS, 128, 2], F32,
                           kind="ExternalOutput").ap()
    aps = (xT4, x_nat, w1r, w2s, w3s, b1s, b2s, b3bc, u_out)
    with tile.TileContext(nc) as tc:
        with ExitStack() as ctx:
            _emit(nc, tc, ctx, aps, B)
    nc.compile()
    return nc


def _prep_core_inputs(x_shard, W1, b1, W21, b21, W22, b22, W31, b31, W32, b32):
    Bc = x_shard.shape[0]
    T = Bc // 512
    xs = np.ascontiguousarray(x_shard, dtype=np.float32).reshape(
        128, T, 4, 4)  # [p, t, j, feat]
    xT4 = np.ascontiguousarray(
        xs.transpose(3, 1, 2, 0).reshape(4, Bc))

    w1r = np.zeros((128, 512), dtype=np.float32)
    for c in range(4):
        w1r[32 * c:32 * c + 4, :] = W1.T

    # W2 fp8 DoubleRow stationary: [k, o, kp, m, i, c] =
    #   W2o[m*64 + c, (2*kp + i)*128 + k]
    w2f = np.stack([W21, W22], axis=0)  # [o, 128, 512]
    w2s = w2f.reshape(2, 2, 64, 4, 128).transpose(4, 0, 3, 1, 2)  # k,o,kb,m,c
    w2s = w2s.reshape(128, 2, 2, 2, 2, 64)  # k, o, kp, i, m, c
    w2s = np.ascontiguousarray(w2s.transpose(0, 1, 2, 4, 3, 5))  # k,o,kp,m,i,c
    w2s = w2s.astype(ml_dtypes.float8_e4m3)

    # w3s[q, o, m, :]: rows are hidden feature m*64+q of branch o's weights
    w3s = np.zeros((64, 2, 2, 4), dtype=np.float32)
    w3s[:, 0, 0, 0:2] = W31.T[0:64]
    w3s[:, 0, 1, 0:2] = W31.T[64:128]
    w3s[:, 1, 0, 2:4] = W32.T[0:64]
    w3s[:, 1, 1, 2:4] = W32.T[64:128]

    b1s = np.ascontiguousarray(b1.reshape(4, 128).T)
    # b2s[q, m, o] = b2o[m*64 + q]
    b2s = np.ascontiguousarray(
        np.stack([b21, b22], axis=1).reshape(2, 64, 2).transpose(1, 0, 2))
    b3bc = np.tile(np.concatenate([b31, b32])[None, :].astype(np.float32),
                   (128, 1))

    return {
        "xT4": xT4,
        "x_nat": np.ascontiguousarray(x_shard, dtype=np.float32).reshape(
            128, Bc // (128 * 128), 128, 4),
        "w1r": w1r,
        "w2s": w2s,
        "w3s": w3s,
        "b1s": b1s,
        "b2s": b2s,
        "b3bc": b3bc,
    }


def kernel(x, W1, b1, W21, b21, W22, b22, W31, b31, W32, b32, sgn=None):
    x = np.asarray(x, dtype=np.float32)
    args = [np.asarray(a, dtype=np.float32)
            for a in (W1, b1, W21, b21, W22, b22, W31, b31, W32, b32)]

    if "nc" not in _CACHE:
        _CACHE["nc"] = _build_kernel(N_CORES, B)
    nc = _CACHE["nc"]

    in_maps = []
    for c in range(N_CORES):
        shard = x[c * B:(c + 1) * B]
        in_maps.append(_prep_core_inputs(shard, *args))

    res = run_bass_kernel_spmd(nc, in_maps, core_ids=list(range(N_CORES)))
    out = np.empty((NB, 2), dtype=np.float32)
    for c in range(N_CORES):
        out[c * B:(c + 1) * B] = res.results[c]["u_out"].reshape(B, 2)
    return out


# revision 19
# speedup vs baseline: 1.3809x; 1.3809x over previous
"""BarrierNet (MLP 4->512->{128,128}->{2,2} + closed-form QP) on 8 Trainium2 cores.

Data-parallel: batch 262144 sharded 8 x 32768; weights replicated.

Per-core layout: sample s = p*256 + 4t + j (p = SBUF partition, t = 512-sample
tile, j = 0..3). The MLP runs feature-major (batch on the PE free dim):

L1 is fp32r with K=5 (ones row folds b1 into the matmul) writing two
[128,2,512] PSUM tiles; each is drained by a single big ACT/DVE op into h1
as fp8e4. L2 splits by branch: h21 runs four plain fp8 matmuls (1 cyc/row,
output on all 128 partitions -> one 512-col drain), h22 runs fp8 DoubleRow
(2 K-tiles per instruction, 0.5 cyc/row; output restricted to PSUM
partitions 0:64, hidden block m in bank m) with per-m merged drains. h2
stays fp32. L3 is flipped: h2 sample-chunks are the *stationary* operand and
the stacked W3 columns the moving [*, 4] operand, so every matmul writes
[128 samples, 4] sample-major into a per-sp PSUM bank (memset + start=False
accumulation) - no transposes, no staging, no DRAM bounce. The QP (sin/cos
via range-reduced ACT Sin, sigmoid, one reciprocal) runs as [128, 128] ops
per sp half mostly on Pool, reading x_nat which is sample-major by
construction. QP ops are emitted as closure lists drained a few per pipeline
slot so no engine sees a burst that would stall the per-tile critical path.

Engine budget per 512-sample tile (ns): PE 2213 (L1 2048c + L2 3072c + L3
192c), ACT ~2100 (h1-A merged drain, h21 drain, h22 share), DVE ~2100 (h1-B
merged drain, h22 share, QP), Pool ~1000 (QP), DMA ~100.
"""
import numpy as np
from contextlib import ExitStack

import ml_dtypes

import concourse.bass as bass
from concourse import bacc as bacc_mod
import concourse.tile as tile
from concourse import mybir
from concourse.bass_utils import run_bass_kernel_spmd

F32 = mybir.dt.float32
F32R = mybir.dt.float32r
F8 = mybir.dt.float8e4
AF = mybir.ActivationFunctionType
OP = mybir.AluOpType
DR = mybir.MatmulPerfMode.DoubleRow

MAGIC = float(np.float32(1.5 * 2 ** 23))
INV2PI = float(np.float32(1.0 / (2 * np.pi)))
TWOPI = float(np.float32(2 * np.pi))
HALFPI = float(np.float32(np.pi / 2))

N_CORES = 8
NB = 262144
B = NB // N_CORES  # 32768 per core

_CACHE = {}


def _emit(nc, tc, ctx, aps, B):
    (xT5, x_nat, w1r, w21s, w22s, w3s, b2s, b22h, b3bc, u_out) = aps
    T = B // 512           # 64 sample tiles per core
    NS = T // 32           # 2 QP half-batches ("sp")

    const = ctx.enter_context(tc.tile_pool(name="const", bufs=1))
    lp = ctx.enter_context(tc.tile_pool(name="lp", bufs=1))
    qp = ctx.enter_context(tc.tile_pool(name="qp", bufs=1))
    ps = ctx.enter_context(tc.tile_pool(name="ps", bufs=1, space="PSUM"))

    w1_sb = const.tile([128, 512], F32R, name="w1_sb", tag="w1_sb")
    nc.sync.dma_start(w1_sb[:], w1r[:])
    w21_sb = const.tile([128, 4, 128], F8, name="w21_sb", tag="w21_sb")
    nc.sync.dma_start(w21_sb[:], w21s[:])
    w22_sb = const.tile([128, 2, 2, 2, 64], F8, name="w22_sb", tag="w22_sb")
    nc.sync.dma_start(w22_sb[:], w22s[:])
    # w3_sb[:, 0, :] = [W31.T | 0] (K=128); w3_sb[0:64, 1+m, :] = [0 | W32.T]
    # for hidden half m (K=64).
    w3_sb = const.tile([128, 3, 4], F32R, name="w3_sb", tag="w3_sb")
    nc.sync.dma_start(w3_sb[:], w3s[:])
    b2_sb = const.tile([128, 2], F32, name="b2_sb", tag="b2_sb")
    nc.sync.dma_start(b2_sb[:], b2s[:])
    # b22h[q, m] = b22[m*64 + q] (lane-aligned bias for the DR m-banks)
    b22h_sb = const.tile([64, 2], F32, name="b22h_sb", tag="b22h_sb")
    nc.sync.dma_start(b22h_sb[:], b22h[:])
    b3_sb = const.tile([128, 4], F32, name="b3_sb", tag="b3_sb")
    nc.sync.dma_start(b3_sb[:], b3bc[:])
    xn_sb = const.tile([128, NS, 128, 4], F32, name="xn_sb", tag="xn_sb")
    nc.sync.dma_start(xn_sb[:], x_nat[:])
    # X5: tile t lives at partition base 32*(t%4), columns 512*(t//4)..
    x4_sb = const.tile([128, T // 4, 512], F32R, name="x4_sb", tag="x4_sb")
    xv = xT5.rearrange("q (tt c r) -> q tt c r", c=4, r=512)
    for c in range(4):
        nc.sync.dma_start(x4_sb[32 * c:32 * c + 5, :, :], xv[:, :, c, :])

    # software-pipelined: window w runs L1(w), L2(w-1), L3(w-2), QP spread
    h1s = {}
    h2s = {}
    ps3s = {}
    geo = {}
    pend = []  # queue of QP op closures, drained a few per slot
    for w in range(T + 4):
        if w < T:
            t = w
            cb = 32 * (t % 4)
            xc = x4_sb[cb:cb + 5, t // 4, :]
            h1 = lp.tile([128, 4, 512], F8, name="h1", tag="h1", bufs=3)
            h1s[t] = h1
            psA = ps.tile([128, 2, 512], F32, name="psA", tag="psA", bufs=1)
            for f in range(2):
                nc.tensor.matmul(
                    psA[:, f, :], w1_sb[cb:cb + 5, 128 * f:128 * (f + 1)],
                    xc, start=True, stop=True, tile_position=(cb, 0))
            nc.scalar.activation(h1[:, 0:2, :], psA[:], AF.Relu)
            psB = ps.tile([128, 2, 512], F32, name="psB", tag="psB", bufs=1)
            for f in range(2):
                nc.tensor.matmul(
                    psB[:, f, :],
                    w1_sb[cb:cb + 5, 128 * (2 + f):128 * (3 + f)],
                    xc, start=True, stop=True, tile_position=(cb, 0))
            nc.scalar.activation(h1[:, 2:4, :], psB[:], AF.Relu)
        if 1 <= w <= T:
            t = w - 1
            h1 = h1s.pop(t)
            # h21 branch: plain fp8 matmuls, out on all 128 partitions.
            ps21 = ps.tile([128, 512], F32, name="ps21", tag="ps21", bufs=1)
            for k in range(4):
                nc.tensor.matmul(ps21[:], w21_sb[:, k, :], h1[:, k, :],
                                 start=(k == 0), stop=(k == 3))
            h21 = lp.tile([128, 512], F32R, name="h21", tag="h21", bufs=2)
            nc.vector.tensor_scalar(h21[:], ps21[:], b2_sb[:, 0:1], 0.0,
                                    op0=OP.add, op1=OP.max)
            # h22 branch: fp8 DoubleRow, out on PSUM partitions 0:64,
            # hidden block m in bank m; h22 stays [64, m, samples].
            ps22 = ps.tile([64, 2, 2, 256], F32, name="ps22", tag="ps22",
                           bufs=1)
            for hh in range(2):
                for m in range(2):
                    for kp in range(2):
                        nc.tensor.matmul(
                            ps22[0:64, m, hh, :],
                            w22_sb[:, kp, m, :, :],
                            h1[:, 2 * kp:2 * kp + 2,
                               256 * hh:256 * hh + 256],
                            start=(hh == 0 and kp == 0), stop=(kp == 1),
                            perf_mode=DR, skip_group_check=True)
            h22 = lp.tile([64, 2, 512], F32R, name="h22", tag="h22", bufs=2)
            h22v = h22[:].rearrange("q m (hh n) -> q m hh n", hh=2)
            for m in range(2):
                nc.vector.tensor_scalar(h22v[:, m, :, :], ps22[0:64, m, :, :],
                                        b22h_sb[:, m:m + 1], 0.0,
                                        op0=OP.add, op1=OP.max)
            h2s[t] = (h21, h22)
        if 2 <= w <= T + 1:
            t = w - 2
            sp, g = divmod(t, 32)
            if g == 0:
                ps3 = ps.tile([128, 128, 4], F32, name="ps3", tag="ps3",
                              bufs=1)
                ps3s[sp] = ps3
                nc.vector.memset(ps3[:], 0.0)
                geo[sp] = {}
                pend.extend(_qp_geo_ops(nc, qp, xn_sb, sp, geo[sp]))
            ps3 = ps3s[sp]
            h21, h22 = h2s.pop(t)
            for j in range(4):
                mi = 4 * g + j
                nc.tensor.matmul(
                    ps3[:, mi, :], h21[:, 128 * j:128 * (j + 1)],
                    w3_sb[:, 0, :], start=False, stop=False,
                    skip_group_check=True)
                for m in range(2):
                    nc.tensor.matmul(
                        ps3[:, mi, :], h22[0:64, m, 128 * j:128 * (j + 1)],
                        w3_sb[0:64, 1 + m, :], start=False,
                        stop=(g == 31 and j == 3 and m == 1),
                        skip_group_check=True)
            if g == 31:
                s3 = qp.tile([128, 128, 4], F32, name="s3", tag="s3", bufs=2)
                nc.scalar.copy(s3[:], ps3s.pop(sp)[:])
                pend.extend(
                    _qp_rest_ops(nc, qp, s3, b3_sb, u_out, sp, geo.pop(sp)))
        # drain a few pending QP ops per slot to avoid engine bursts
        for _ in range(3):
            if pend:
                pend.pop(0)()
    while pend:
        pend.pop(0)()


def _qp_tile(nc, qp, name, bufs=1):
    return qp.tile([128, 128], F32, name=name, tag=name, bufs=bufs)


def _qp_geo_ops(nc, qp, xn_sb, sp, out):
    """x-only QP quantities (no MLP dependency) as a list of op closures.

    Results consumed by _qp_rest_ops (next sp window) use bufs=2 tiles.
    """
    r = {}

    def tt(name, a, b, op, eng=None, bufs=1):
        def f():
            o = _qp_tile(nc, qp, name, bufs=bufs)
            (eng or nc.gpsimd).tensor_tensor(o[:], r[a], r[b], op=op)
            r[name] = o[:]
            out[name] = o[:]
        return f

    def ts(name, a, s1, op0, s2=None, op1=None, eng=None, bufs=1):
        def f():
            o = _qp_tile(nc, qp, name, bufs=bufs)
            if s2 is None:
                (eng or nc.vector).tensor_scalar(o[:], r[a], s1, None, op0=op0)
            else:
                (eng or nc.vector).tensor_scalar(o[:], r[a], s1, s2,
                                                 op0=op0, op1=op1)
            r[name] = o[:]
            out[name] = o[:]
        return f

    def act(name, a, func):
        def f():
            o = _qp_tile(nc, qp, name)
            nc.scalar.activation(o[:], r[a], func)
            r[name] = o[:]
            out[name] = o[:]
        return f

    r["PX"] = xn_sb[:, sp, :, 0]
    r["PY"] = xn_sb[:, sp, :, 1]
    r["TH"] = xn_sb[:, sp, :, 2]
    r["VV"] = xn_sb[:, sp, :, 3]

    P = nc.gpsimd
    return [fn for fn in [
        ts("dx", "PX", 10.0, OP.mult, 10.0, OP.add),
        ts("dy", "PY", 10.0, OP.mult, 5.0, OP.add),
        ts("v", "VV", 2.0, OP.mult, 5.0, OP.add),
        ts("f1", "TH", INV2PI, OP.mult, MAGIC, OP.add),
        ts("f2", "f1", MAGIC, OP.subtract, TWOPI, OP.mult),
        tt("thr", "TH", "f2", OP.subtract, eng=P),
        act("st", "thr", AF.Sin),
        ts("c1", "TH", INV2PI, OP.mult, 0.25, OP.add),
        ts("c2", "c1", MAGIC, OP.add, MAGIC, OP.subtract),
        ts("c3", "c2", TWOPI, OP.mult, HALFPI, OP.subtract),
        tt("thc", "TH", "c3", OP.subtract, eng=P),
        act("ct", "thc", AF.Sin),
        tt("vst", "v", "st", OP.mult, eng=P),
        tt("vct", "v", "ct", OP.mult, eng=P),
        tt("dx2", "dx", "dx", OP.mult, eng=P),
        tt("dy2", "dy", "dy", OP.mult, eng=P),
        tt("bar", "dx2", "dy2", OP.add, eng=P),
        ts("bar16", "bar", 16.0, OP.mult, 576.0, OP.subtract, bufs=2),
        tt("a3", "dx", "vct", OP.mult, eng=P),
        tt("a4", "dy", "vst", OP.mult, eng=P),
        tt("a5", "a3", "a4", OP.add, eng=P),
        ts("bdot4", "a5", 8.0, OP.mult, bufs=2),
        tt("v2", "v", "v", OP.mult, eng=P),
        ts("v22", "v2", 2.0, OP.mult, bufs=2),
        tt("g1a", "dx", "vst", OP.mult, eng=P),
        tt("g1b", "dy", "vct", OP.mult, eng=P),
        tt("G1p", "g1a", "g1b", OP.subtract, eng=P, bufs=2),
        tt("g2a", "dx", "ct", OP.mult, eng=P),
        tt("g2b", "dy", "st", OP.mult, eng=P),
        tt("G2pp", "g2a", "g2b", OP.add, eng=P, bufs=2),
        tt("q1", "G1p", "G1p", OP.mult, eng=P),
        tt("q2", "G2pp", "G2pp", OP.mult, eng=P),
        tt("q3", "q1", "q2", OP.add, eng=P),
        ts("ggc", "q3", 4.0, OP.mult, 1e-12, OP.max),
        _recip(nc, qp, r, out),
    ] if fn is not None]


def _recip(nc, qp, r, out):
    def f():
        o = _qp_tile(nc, qp, "rec", bufs=2)
        nc.vector.reciprocal(o[:], r["ggc"])
        out["rec"] = o[:]
    return f


def _qp_rest_ops(nc, qp, s3, b3_sb, u_out, sp, g):
    r = dict(g)

    def tt(name, a, b, op, eng=None):
        def f():
            o = _qp_tile(nc, qp, name)
            (eng or nc.gpsimd).tensor_tensor(o[:], r[a], r[b], op=op)
            r[name] = o[:]
        return f

    def stt(name, a, s, b, op0, op1, eng=None):
        def f():
            o = _qp_tile(nc, qp, name)
            (eng or nc.vector).scalar_tensor_tensor(o[:], r[a], s, r[b],
                                                    op0=op0, op1=op1)
            r[name] = o[:]
        return f

    def sig(name, src, bcol):
        def f():
            o = _qp_tile(nc, qp, name)
            nc.scalar.activation(o[:], src, AF.Sigmoid,
                                 bias=b3_sb[:, bcol:bcol + 1])
            r[name] = o[:]
        return f

    def addb(name, src, bcol):
        def f():
            o = _qp_tile(nc, qp, name)
            nc.vector.tensor_scalar(o[:], src, b3_sb[:, bcol:bcol + 1], None,
                                    op0=OP.add)
            r[name] = o[:]
        return f

    X31A = s3[:, :, 0]
    X31B = s3[:, :, 1]
    Z32A = s3[:, :, 2]
    Z32B = s3[:, :, 3]

    def emit_u():
        u_sb = qp.tile([128, 128, 2], F32, name="u_sb", tag="u_sb", bufs=2)
        nc.gpsimd.tensor_tensor(u_sb[:, :, 0], r["z1"], r["xa"],
                                op=OP.subtract)
        nc.vector.tensor_tensor(u_sb[:, :, 1], r["z2"], r["xb"],
                                op=OP.subtract)
        nc.sync.dma_start(u_out[:, sp, :, :], u_sb[:])

    return [
        sig("sa", Z32A, 2),
        sig("sb_", Z32B, 3),
        addb("xa", X31A, 0),
        addb("xb", X31B, 1),
        tt("ssum", "sa", "sb_", OP.add),
        tt("sprod", "sa", "sb_", OP.mult),
        tt("m1", "ssum", "bdot4", OP.mult),
        tt("m2", "sprod", "bar16", OP.mult),
        tt("m3", "m1", "m2", OP.add),
        tt("h", "v22", "m3", OP.add),
        tt("n1", "G1p", "xa", OP.mult),
        tt("n2", "G2pp", "xb", OP.mult),
        tt("n3", "n1", "n2", OP.subtract),
        stt("num", "n3", -2.0, "h", OP.mult, OP.subtract),
        stt("lam", "num", 0.0, "rec", OP.max, OP.mult),
        stt("z1", "lam", -2.0, "G1p", OP.mult, OP.mult),
        stt("z2", "lam", 2.0, "G2pp", OP.mult, OP.mult),
        emit_u,
    ]


def _build_kernel(n_cores, B):
    nc = bacc_mod.Bacc("TRN2", target_bir_lowering=False, debug=False,
                       num_devices=n_cores)
    NS = (B // 512) // 32
    xT5 = nc.dram_tensor("xT5", [5, B], F32R, kind="ExternalInput").ap()
    x_nat = nc.dram_tensor("x_nat", [128, NS, 128, 4], F32,
                           kind="ExternalInput").ap()
    w1r = nc.dram_tensor("w1r", [128, 512], F32R, kind="ExternalInput").ap()
    w21s = nc.dram_tensor("w21s", [128, 4, 128], F8,
                          kind="ExternalInput").ap()
    w22s = nc.dram_tensor("w22s", [128, 2, 2, 2, 64], F8,
                          kind="ExternalInput").ap()
    w3s = nc.dram_tensor("w3s", [128, 3, 4], F32R, kind="ExternalInput").ap()
    b2s = nc.dram_tensor("b2s", [128, 2], F32, kind="ExternalInput").ap()
    b22h = nc.dram_tensor("b22h", [64, 2], F32, kind="ExternalInput").ap()
    b3bc = nc.dram_tensor("b3bc", [128, 4], F32, kind="ExternalInput").ap()
    u_out = nc.dram_tensor("u_out", [128, NS, 128, 2], F32,
                           kind="ExternalOutput").ap()
    aps = (xT5, x_nat, w1r, w21s, w22s, w3s, b2s, b22h, b3bc, u_out)
    with tile.TileContext(nc) as tc:
        with ExitStack() as ctx:
            _emit(nc, tc, ctx, aps, B)
    nc.compile()
    return nc


def _prep_core_inputs(x_shard, W1, b1, W21, b21, W22, b22, W31, b31, W32, b32):
    Bc = x_shard.shape[0]
    T = Bc // 512
    xs = np.ascontiguousarray(x_shard, dtype=np.float32).reshape(
        128, T, 4, 4)  # [p, t, j, feat]
    xT5 = np.empty((5, Bc), dtype=np.float32)
    xT5[:4] = xs.transpose(3, 1, 2, 0).reshape(4, Bc)
    xT5[4] = 1.0

    w1r = np.zeros((128, 512), dtype=np.float32)
    w1e = np.concatenate([W1.T, b1[None, :]], axis=0)
    for c in range(4):
        w1r[32 * c:32 * c + 5, :] = w1e

    # h21 branch: plain fp8 stationary [k, kb, c] = W21[c, kb*128 + k]
    w21s = np.ascontiguousarray(
        W21.T.reshape(4, 128, 128).transpose(1, 0, 2)
    ).astype(ml_dtypes.float8_e4m3)

    # h22 branch DoubleRow stationary: [k, kp, m, i, c] =
    #   W22[m*64 + c, (2*kp + i)*128 + k]
    w22s = W22.reshape(2, 64, 4, 128).transpose(3, 2, 0, 1)  # k, kb, m, c
    w22s = w22s.reshape(128, 2, 2, 2, 64)  # k, kp, i, m, c
    w22s = np.ascontiguousarray(w22s.transpose(0, 1, 3, 2, 4))  # k,kp,m,i,c
    w22s = w22s.astype(ml_dtypes.float8_e4m3)

    w3s = np.zeros((128, 3, 4), dtype=np.float32)
    w3s[:, 0, 0:2] = W31.T
    w3s[0:64, 1, 2:4] = W32.T[0:64]
    w3s[0:64, 2, 2:4] = W32.T[64:128]

    b2s = np.stack([b21, b22], axis=1).astype(np.float32)
    b22h = np.ascontiguousarray(b22.reshape(2, 64).T.astype(np.float32))
    b3bc = np.tile(np.concatenate([b31, b32])[None, :].astype(np.float32),
                   (128, 1))

    return {
        "xT5": xT5,
        "x_nat": np.ascontiguousarray(x_shard, dtype=np.float32).reshape(
            128, Bc // (128 * 128), 128, 4),
        "w1r": w1r,
        "w21s": w21s,
        "w22s": w22s,
        "w3s": w3s,
        "b2s": b2s,
        "b22h": b22h,
        "b3bc": b3bc,
    }


def kernel(x, W1, b1, W21, b21, W22, b22, W31, b31, W32, b32, sgn=None):
    x = np.asarray(x, dtype=np.float32)
    args = [np.asarray(a, dtype=np.float32)
            for a in (W1, b1, W21, b21, W22, b22, W31, b31, W32, b32)]

    if "nc" not in _CACHE:
        _CACHE["nc"] = _build_kernel(N_CORES, B)
    nc = _CACHE["nc"]

    in_maps = []
    for c in range(N_CORES):
        shard = x[c * B:(c + 1) * B]
        in_maps.append(_prep_core_inputs(shard, *args))

    res = run_bass_kernel_spmd(nc, in_maps, core_ids=list(range(N_CORES)))
    out = np.empty((NB, 2), dtype=np.float32)
    for c in range(N_CORES):
        out[c * B:(c + 1) * B] = res.results[c]["u_out"].reshape(B, 2)
    return out


# revision 26
# speedup vs baseline: 1.5178x; 1.0992x over previous
"""BarrierNet (MLP 4->512->{128,128}->{2,2} + closed-form QP) on 8 Trainium2 cores.

Data-parallel: batch 262144 sharded 8 x 32768; weights replicated.

Per-core layout: sample s = p*256 + 4t + j (p = SBUF partition, t = 512-sample
tile, j = 0..3). The MLP runs feature-major (batch on the PE free dim):

L1 is fp32r with K=5 (ones row folds b1 into the matmul) writing two
[128,2,512] PSUM tiles; each is drained by a single big ACT/DVE op into h1
as fp8e4. L2 splits by branch: h21 runs four plain fp8 matmuls (1 cyc/row,
output on all 128 partitions -> one 512-col drain), h22 runs fp8 DoubleRow
(2 K-tiles per instruction, 0.5 cyc/row; output restricted to PSUM
partitions 0:64, hidden block m in bank m) with per-m merged drains. h2
stays fp32. L3 is flipped: h2 sample-chunks are the *stationary* operand and
the stacked W3 columns the moving [*, 4] operand, so every matmul writes
[128 samples, 4] sample-major into a per-sp PSUM bank (memset + start=False
accumulation) - no transposes, no staging, no DRAM bounce. The QP (sin/cos
via range-reduced ACT Sin, sigmoid, one reciprocal) runs as [128, 128] ops
per sp half mostly on Pool, reading x_nat which is sample-major by
construction. QP ops are emitted as closure lists drained a few per pipeline
slot so no engine sees a burst that would stall the per-tile critical path.

Engine budget per 512-sample tile (ns): PE 2213 (L1 2048c + L2 3072c + L3
192c), ACT ~2100 (h1-A merged drain, h21 drain, h22 share), DVE ~2100 (h1-B
merged drain, h22 share, QP), Pool ~1000 (QP), DMA ~100.
"""
import numpy as np
from contextlib import ExitStack

import ml_dtypes

import concourse.bass as bass
from concourse import bacc as bacc_mod
import concourse.tile as tile
from concourse import mybir
from concourse.bass_utils import run_bass_kernel_spmd

F32 = mybir.dt.float32
F32R = mybir.dt.float32r
F8 = mybir.dt.float8e4
AF = mybir.ActivationFunctionType
OP = mybir.AluOpType
DR = mybir.MatmulPerfMode.DoubleRow

MAGIC = float(np.float32(1.5 * 2 ** 23))
INV2PI = float(np.float32(1.0 / (2 * np.pi)))
TWOPI = float(np.float32(2 * np.pi))
HALFPI = float(np.float32(np.pi / 2))

N_CORES = 8
NB = 262144
B = NB // N_CORES  # 32768 per core

_CACHE = {}


def _emit(nc, tc, ctx, aps, B):
    (xT5, x_nat, w1r, w21s, w22s, w3s, b2s, b22h, b3bc, b3f, u_out) = aps
    T = B // 512           # 64 sample tiles per core
    NS = T // 32           # 2 QP half-batches ("sp")

    const = ctx.enter_context(tc.tile_pool(name="const", bufs=1))
    lp = ctx.enter_context(tc.tile_pool(name="lp", bufs=1))
    qp = ctx.enter_context(tc.tile_pool(name="qp", bufs=1))
    ps = ctx.enter_context(tc.tile_pool(name="ps", bufs=1, space="PSUM"))

    # DMA order matters: transfers serialize on the HWDGE/DMA devices, so
    # stage exactly what the pipeline needs first (L1 tile 0, then L2
    # weights, then the remaining x tiles, then QP-only data).
    w1_sb = const.tile([128, 512], F32R, name="w1_sb", tag="w1_sb")
    nc.sync.dma_start(w1_sb[:], w1r[:])
    x4_sb = const.tile([128, T // 4, 512], F32R, name="x4_sb", tag="x4_sb")
    xv = xT5.rearrange("q (tt c r) -> q tt c r", c=4, r=512)
    # X5: tile t lives at partition base 32*(t%4), columns 512*(t//4)..
    nc.sync.dma_start(x4_sb[0:5, :, :], xv[:, :, 0, :])
    w21_sb = const.tile([128, 4, 128], F8, name="w21_sb", tag="w21_sb")
    nc.sync.dma_start(w21_sb[:], w21s[:])
    w22_sb = const.tile([128, 2, 2, 2, 64], F8, name="w22_sb", tag="w22_sb")
    nc.sync.dma_start(w22_sb[:], w22s[:])
    b2_sb = const.tile([128, 2], F32, name="b2_sb", tag="b2_sb")
    nc.sync.dma_start(b2_sb[:], b2s[:])
    # b22h[q, m] = b22[m*64 + q] (lane-aligned bias for the DR m-banks)
    b22h_sb = const.tile([64, 2], F32, name="b22h_sb", tag="b22h_sb")
    nc.sync.dma_start(b22h_sb[:], b22h[:])
    for c in range(1, 4):
        nc.sync.dma_start(x4_sb[32 * c:32 * c + 5, :, :], xv[:, :, c, :])
    # w3_sb[:, 0, :] = [W31.T | 0] (K=128); w3_sb[0:64, 1+m, :] = [0 | W32.T]
    # for hidden half m (K=64).
    w3_sb = const.tile([128, 3, 4], F32R, name="w3_sb", tag="w3_sb")
    nc.sync.dma_start(w3_sb[:], w3s[:])
    xn_sb = const.tile([128, NS, 128, 4], F32, name="xn_sb", tag="xn_sb")
    nc.sync.dma_start(xn_sb[:], x_nat[:])
    b3_sb = const.tile([128, 4], F32, name="b3_sb", tag="b3_sb")
    nc.sync.dma_start(b3_sb[:], b3bc[:])
    # b3f[:, i, :]: b31[i] broadcast, so Pool (TensorTensor-only) can add it
    b3f_sb = const.tile([128, 2, 128], F32, name="b3f_sb", tag="b3f_sb")
    nc.sync.dma_start(b3f_sb[:], b3f[:])
    # broadcast constants for Pool-side affine ops
    cst = {}
    for val in (2.0, 5.0, 8.0, 10.0, 16.0, 576.0):
        cst[val] = const.tile([128, 128], F32, name=f"c{int(val)}",
                              tag=f"c{int(val)}")
        nc.gpsimd.memset(cst[val][:], val)

    # software-pipelined: window w runs L1(w), L2(w-1), L3(w-2), QP spread
    h1s = {}
    h2s = {}
    ps3s = {}
    geo = {}
    pend = []  # queue of QP op closures, drained a few per slot
    for w in range(T + 4):
        if w < T:
            t = w
            cb = 32 * (t % 4)
            xc = x4_sb[cb:cb + 5, t // 4, :]
            h1 = lp.tile([128, 4, 512], F8, name="h1", tag="h1", bufs=3)
            h1s[t] = h1
            psA = ps.tile([128, 2, 512], F32, name="psA", tag="psA", bufs=1)
            for f in range(2):
                nc.tensor.matmul(
                    psA[:, f, :], w1_sb[cb:cb + 5, 128 * f:128 * (f + 1)],
                    xc, start=True, stop=True, tile_position=(cb, 0))
            nc.scalar.activation(h1[:, 0:2, :], psA[:], AF.Relu)
            psB = ps.tile([128, 2, 512], F32, name="psB", tag="psB", bufs=1)
            for f in range(2):
                nc.tensor.matmul(
                    psB[:, f, :],
                    w1_sb[cb:cb + 5, 128 * (2 + f):128 * (3 + f)],
                    xc, start=True, stop=True, tile_position=(cb, 0))
            nc.scalar.activation(h1[:, 2:4, :], psB[:], AF.Relu)
        if 1 <= w <= T:
            t = w - 1
            h1 = h1s.pop(t)
            # h21 branch: plain fp8 matmuls, out on all 128 partitions.
            ps21 = ps.tile([128, 512], F32, name="ps21", tag="ps21", bufs=1)
            for k in range(4):
                nc.tensor.matmul(ps21[:], w21_sb[:, k, :], h1[:, k, :],
                                 start=(k == 0), stop=(k == 3))
            h21 = lp.tile([128, 512], F32R, name="h21", tag="h21", bufs=2)
            nc.vector.tensor_scalar(h21[:], ps21[:], b2_sb[:, 0:1], 0.0,
                                    op0=OP.add, op1=OP.max)
            # h22 branch: fp8 DoubleRow, out on PSUM partitions 0:64,
            # hidden block m in bank m; h22 stays [64, m, samples].
            ps22 = ps.tile([64, 2, 2, 256], F32, name="ps22", tag="ps22",
                           bufs=1)
            for hh in range(2):
                for m in range(2):
                    for kp in range(2):
                        nc.tensor.matmul(
                            ps22[0:64, m, hh, :],
                            w22_sb[:, kp, m, :, :],
                            h1[:, 2 * kp:2 * kp + 2,
                               256 * hh:256 * hh + 256],
                            start=(hh == 0 and kp == 0), stop=(kp == 1),
                            perf_mode=DR, skip_group_check=True)
            h22 = lp.tile([64, 2, 512], F32R, name="h22", tag="h22", bufs=2)
            h22v = h22[:].rearrange("q m (hh n) -> q m hh n", hh=2)
            for m in range(2):
                nc.vector.tensor_scalar(h22v[:, m, :, :], ps22[0:64, m, :, :],
                                        b22h_sb[:, m:m + 1], 0.0,
                                        op0=OP.add, op1=OP.max)
            h2s[t] = (h21, h22)
        if 2 <= w <= T + 1:
            t = w - 2
            sp, g = divmod(t, 32)
            if g == 0:
                ps3 = ps.tile([128, 128, 4], F32, name="ps3", tag="ps3",
                              bufs=1)
                ps3s[sp] = ps3
                nc.vector.memset(ps3[:], 0.0)
                geo[sp] = {}
                pend.extend(_qp_geo_ops(nc, qp, xn_sb, sp, geo[sp], cst))
            ps3 = ps3s[sp]
            h21, h22 = h2s.pop(t)
            for j in range(4):
                mi = 4 * g + j
                nc.tensor.matmul(
                    ps3[:, mi, :], h21[:, 128 * j:128 * (j + 1)],
                    w3_sb[:, 0, :], start=False, stop=False,
                    skip_group_check=True)
                for m in range(2):
                    nc.tensor.matmul(
                        ps3[:, mi, :], h22[0:64, m, 128 * j:128 * (j + 1)],
                        w3_sb[0:64, 1 + m, :], start=False,
                        stop=(g == 31 and j == 3 and m == 1),
                        skip_group_check=True)
            if g in (15, 31):
                # QP tail chunk: half the sp as soon as its L3 rows landed
                c = g // 16
                s3c = qp.tile([128, 64, 4], F32, name="s3", tag="s3", bufs=2)
                nc.scalar.copy(s3c[:], ps3[:, 64 * c:64 * c + 64, :])
                pend.extend(_qp_rest_ops(nc, qp, s3c, b3f_sb, b3_sb, u_out,
                                         sp, c, geo[sp]))
                if g == 31:
                    ps3s.pop(sp)
        # drain a few pending QP ops per slot to avoid engine bursts; hold
        # off at the start so an xn_sb-gated op can't block a drain stream
        if w >= 6:
            for _ in range(3):
                if pend:
                    pend.pop(0)()
    while pend:
        pend.pop(0)()


def _qp_tile(nc, qp, name, bufs=1):
    return qp.tile([128, 128], F32, name=name, tag=name, bufs=bufs)


def _qp_geo_ops(nc, qp, xn_sb, sp, out, cst):
    """x-only QP quantities (no MLP dependency) as a list of op closures.

    Almost everything runs on Pool (TensorTensor-only engine) using the
    broadcast const tiles; DVE keeps only the round-trip "magic" ops whose
    intermediate rounding an affine op can't reproduce, plus the
    reciprocal. Results consumed by _qp_rest_ops use bufs=2 tiles.
    """
    r = {}

    def tt(name, a, b, op, bufs=1):
        def f():
            o = _qp_tile(nc, qp, name, bufs=bufs)
            bb = cst[b][:] if isinstance(b, float) else r[b]
            nc.gpsimd.tensor_tensor(o[:], r[a], bb, op=op)
            r[name] = o[:]
            out[name] = o[:]
        return f

    def ts(name, a, s1, op0, s2=None, op1=None, bufs=1):
        def f():
            o = _qp_tile(nc, qp, name, bufs=bufs)
            if s2 is None:
                nc.vector.tensor_scalar(o[:], r[a], s1, None, op0=op0)
            else:
                nc.vector.tensor_scalar(o[:], r[a], s1, s2, op0=op0, op1=op1)
            r[name] = o[:]
            out[name] = o[:]
        return f

    def act(name, a, func):
        def f():
            o = _qp_tile(nc, qp, name)
            nc.scalar.activation(o[:], r[a], func)
            r[name] = o[:]
            out[name] = o[:]
        return f

    r["PX"] = xn_sb[:, sp, :, 0]
    r["PY"] = xn_sb[:, sp, :, 1]
    r["TH"] = xn_sb[:, sp, :, 2]
    r["VV"] = xn_sb[:, sp, :, 3]

    return [
        tt("dxa", "PX", 10.0, OP.mult),
        ts("f1", "TH", INV2PI, OP.mult, MAGIC, OP.add),
        tt("dx", "dxa", 10.0, OP.add),
        ts("f2", "f1", MAGIC, OP.subtract, TWOPI, OP.mult),
        tt("dya", "PY", 10.0, OP.mult),
        ts("c1", "TH", INV2PI, OP.mult, 0.25, OP.add),
        tt("dy", "dya", 5.0, OP.add),
        tt("thr", "TH", "f2", OP.subtract),
        tt("va", "VV", 2.0, OP.mult),
        ts("c2", "c1", MAGIC, OP.add, MAGIC, OP.subtract),
        tt("v", "va", 5.0, OP.add),
        ts("c3", "c2", TWOPI, OP.mult, HALFPI, OP.subtract),
        act("st", "thr", AF.Sin),
        tt("dx2", "dx", "dx", OP.mult),
        tt("thc", "TH", "c3", OP.subtract),
        tt("dy2", "dy", "dy", OP.mult),
        act("ct", "thc", AF.Sin),
        tt("bar", "dx2", "dy2", OP.add),
        tt("vst", "v", "st", OP.mult),
        tt("bar16a", "bar", 16.0, OP.mult),
        tt("vct", "v", "ct", OP.mult),
        tt("bar16", "bar16a", 576.0, OP.subtract, bufs=2),
        tt("a3", "dx", "vct", OP.mult),
        tt("a4", "dy", "vst", OP.mult),
        tt("a5", "a3", "a4", OP.add),
        tt("bdot4", "a5", 8.0, OP.mult, bufs=2),
        tt("v2", "v", "v", OP.mult),
        tt("v22", "v2", 2.0, OP.mult, bufs=2),
        tt("g1a", "dx", "vst", OP.mult),
        tt("g1b", "dy", "vct", OP.mult),
        tt("G1p", "g1a", "g1b", OP.subtract, bufs=2),
        tt("g2a", "dx", "ct", OP.mult),
        tt("g2b", "dy", "st", OP.mult),
        tt("G2pp", "g2a", "g2b", OP.add, bufs=2),
        tt("q1", "G1p", "G1p", OP.mult),
        tt("q2", "G2pp", "G2pp", OP.mult),
        tt("q3", "q1", "q2", OP.add),
        ts("ggc", "q3", 4.0, OP.mult, 1e-12, OP.max),
        _recip(nc, qp, r, out),
    ]


def _recip(nc, qp, r, out):
    def f():
        o = _qp_tile(nc, qp, "rec", bufs=2)
        nc.vector.reciprocal(o[:], r["ggc"])
        out["rec"] = o[:]
    return f


def _qp_rest_ops(nc, qp, s3c, b3f_sb, b3_sb, u_out, sp, c, g):
    """QP epilogue for sample half-chunk c of sp (columns 64c..64c+64)."""
    r = {}
    csl = slice(64 * c, 64 * c + 64)

    def get(k):
        return r[k] if k in r else g[k][:, csl]

    def tt(name, a, b, op):
        def f():
            o = qp.tile([128, 64], F32, name=name, tag="r_" + name, bufs=1)
            nc.gpsimd.tensor_tensor(o[:], get(a), get(b), op=op)
            r[name] = o[:]
        return f

    def stt(name, a, s, b, op0, op1):
        def f():
            o = qp.tile([128, 64], F32, name=name, tag="r_" + name, bufs=1)
            nc.vector.scalar_tensor_tensor(o[:], get(a), s, get(b),
                                           op0=op0, op1=op1)
            r[name] = o[:]
        return f

    def sig(name, src, bcol):
        def f():
            o = qp.tile([128, 64], F32, name=name, tag="r_" + name, bufs=1)
            nc.scalar.activation(o[:], src, AF.Sigmoid,
                                 bias=b3_sb[:, bcol:bcol + 1])
            r[name] = o[:]
        return f

    def addb(name, src, i):
        def f():
            o = qp.tile([128, 64], F32, name=name, tag="r_" + name, bufs=1)
            nc.gpsimd.tensor_tensor(o[:], src, b3f_sb[:, i, csl], op=OP.add)
            r[name] = o[:]
        return f

    X31A = s3c[:, :, 0]
    X31B = s3c[:, :, 1]
    Z32A = s3c[:, :, 2]
    Z32B = s3c[:, :, 3]

    def emit_u():
        u_sb = qp.tile([128, 64, 2], F32, name="u_sb", tag="u_sb", bufs=2)
        nc.gpsimd.tensor_tensor(u_sb[:, :, 0], r["z1"], r["xa"],
                                op=OP.subtract)
        nc.gpsimd.tensor_tensor(u_sb[:, :, 1], r["z2"], r["xb"],
                                op=OP.subtract)
        nc.sync.dma_start(u_out[:, sp, csl, :], u_sb[:])

    return [
        sig("sa", Z32A, 2),
        addb("xa", X31A, 0),
        sig("sb_", Z32B, 3),
        addb("xb", X31B, 1),
        tt("ssum", "sa", "sb_", OP.add),
        tt("sprod", "sa", "sb_", OP.mult),
        tt("m1", "ssum", "bdot4", OP.mult),
        tt("m2", "sprod", "bar16", OP.mult),
        tt("n1", "G1p", "xa", OP.mult),
        tt("m3", "m1", "m2", OP.add),
        tt("n2", "G2pp", "xb", OP.mult),
        tt("h", "v22", "m3", OP.add),
        tt("n3", "n1", "n2", OP.subtract),
        stt("num", "n3", -2.0, "h", OP.mult, OP.subtract),
        stt("lam", "num", 0.0, "rec", OP.max, OP.mult),
        stt("z1", "lam", -2.0, "G1p", OP.mult, OP.mult),
        stt("z2", "lam", 2.0, "G2pp", OP.mult, OP.mult),
        emit_u,
    ]


def _build_kernel(n_cores, B):
    nc = bacc_mod.Bacc("TRN2", target_bir_lowering=False, debug=False,
                       num_devices=n_cores)
    NS = (B // 512) // 32
    xT5 = nc.dram_tensor("xT5", [5, B], F32R, kind="ExternalInput").ap()
    x_nat = nc.dram_tensor("x_nat", [128, NS, 128, 4], F32,
                           kind="ExternalInput").ap()
    w1r = nc.dram_tensor("w1r", [128, 512], F32R, kind="ExternalInput").ap()
    w21s = nc.dram_tensor("w21s", [128, 4, 128], F8,
                          kind="ExternalInput").ap()
    w22s = nc.dram_tensor("w22s", [128, 2, 2, 2, 64], F8,
                          kind="ExternalInput").ap()
    w3s = nc.dram_tensor("w3s", [128, 3, 4], F32R, kind="ExternalInput").ap()
    b2s = nc.dram_tensor("b2s", [128, 2], F32, kind="ExternalInput").ap()
    b22h = nc.dram_tensor("b22h", [64, 2], F32, kind="ExternalInput").ap()
    b3bc = nc.dram_tensor("b3bc", [128, 4], F32, kind="ExternalInput").ap()
    b3f = nc.dram_tensor("b3f", [128, 2, 128], F32, kind="ExternalInput").ap()
    u_out = nc.dram_tensor("u_out", [128, NS, 128, 2], F32,
                           kind="ExternalOutput").ap()
    aps = (xT5, x_nat, w1r, w21s, w22s, w3s, b2s, b22h, b3bc, b3f, u_out)
    with tile.TileContext(nc) as tc:
        with ExitStack() as ctx:
            _emit(nc, tc, ctx, aps, B)
    nc.compile()
    return nc


def _prep_core_inputs(x_shard, W1, b1, W21, b21, W22, b22, W31, b31, W32, b32):
    Bc = x_shard.shape[0]
    T = Bc // 512
    xs = np.ascontiguousarray(x_shard, dtype=np.float32).reshape(
        128, T, 4, 4)  # [p, t, j, feat]
    xT5 = np.empty((5, Bc), dtype=np.float32)
    xT5[:4] = xs.transpose(3, 1, 2, 0).reshape(4, Bc)
    xT5[4] = 1.0

    w1r = np.zeros((128, 512), dtype=np.float32)
    w1e = np.concatenate([W1.T, b1[None, :]], axis=0)
    for c in range(4):
        w1r[32 * c:32 * c + 5, :] = w1e

    # h21 branch: plain fp8 stationary [k, kb, c] = W21[c, kb*128 + k]
    w21s = np.ascontiguousarray(
        W21.T.reshape(4, 128, 128).transpose(1, 0, 2)
    ).astype(ml_dtypes.float8_e4m3)

    # h22 branch DoubleRow stationary: [k, kp, m, i, c] =
    #   W22[m*64 + c, (2*kp + i)*128 + k]
    w22s = W22.reshape(2, 64, 4, 128).transpose(3, 2, 0, 1)  # k, kb, m, c
    w22s = w22s.reshape(128, 2, 2, 2, 64)  # k, kp, i, m, c
    w22s = np.ascontiguousarray(w22s.transpose(0, 1, 3, 2, 4))  # k,kp,m,i,c
    w22s = w22s.astype(ml_dtypes.float8_e4m3)

    w3s = np.zeros((128, 3, 4), dtype=np.float32)
    w3s[:, 0, 0:2] = W31.T
    w3s[0:64, 1, 2:4] = W32.T[0:64]
    w3s[0:64, 2, 2:4] = W32.T[64:128]

    b2s = np.stack([b21, b22], axis=1).astype(np.float32)
    b22h = np.ascontiguousarray(b22.reshape(2, 64).T.astype(np.float32))
    b3bc = np.tile(np.concatenate([b31, b32])[None, :].astype(np.float32),
                   (128, 1))
    b3f = np.empty((128, 2, 128), dtype=np.float32)
    b3f[:, 0, :] = b31[0]
    b3f[:, 1, :] = b31[1]

    return {
        "xT5": xT5,
        "x_nat": np.ascontiguousarray(x_shard, dtype=np.float32).reshape(
            128, Bc // (128 * 128), 128, 4),
        "w1r": w1r,
        "w21s": w21s,
        "w22s": w22s,
        "w3s": w3s,
        "b2s": b2s,
        "b22h": b22h,
        "b3bc": b3bc,
        "b3f": b3f,
    }


def kernel(x, W1, b1, W21, b21, W22, b22, W31, b31, W32, b32, sgn=None):
    x = np.asarray(x, dtype=np.float32)
    args = [np.asarray(a, dtype=np.float32)
            for a in (W1, b1, W21, b21, W22, b22, W31, b31, W32, b32)]

    if "nc" not in _CACHE:
        _CACHE["nc"] = _build_kernel(N_CORES, B)
    nc = _CACHE["nc"]

    in_maps = []
    for c in range(N_CORES):
        shard = x[c * B:(c + 1) * B]
        in_maps.append(_prep_core_inputs(shard, *args))

    res = run_bass_kernel_spmd(nc, in_maps, core_ids=list(range(N_CORES)))
    out = np.empty((NB, 2), dtype=np.float32)
    for c in range(N_CORES):
        out[c * B:(c + 1) * B] = res.results[c]["u_out"].reshape(B, 2)
    return out
